# revision 1
# baseline (speedup 1.0000x reference)
"""Bass/Trainium2 kernel for blockwise cross-attention.

Math (per batch element b, per 16-row block):
  out1 = softmax(q1 k2^T / sqrt(E)) @ v2,  out2 = softmax(q2 k1^T / sqrt(E)) @ v1
with q = x Wq^T + bq etc.  Since softmax is shift-invariant along the key
axis, the q-side bias terms drop and
  softmax(q1 k2^T / s) == softmax(x1 A x2^T + 1 (x2 c)^T)
with A = Wq^T Wk / s and c = Wk^T bq / s precomputed on the host.  This
replaces 6 big projections with 4 (z = x A^T fused for both q&k roles, plus
v' = x Wv^T).  The v bias folds in exactly because softmax rows sum to 1.

Sharding: pure data-parallel — batch B=8, one batch element per NeuronCore.

Device flow per core (S=4096 rows, E=512), bf16 matmuls / fp32 softmax:
  - x^T tiles [128e, 512rows] per 512-row group (host pre-transposes, bf16)
  - z^T = A x^T via stationary A^T chunks; t = x c via stationary c chunks;
    v' = x Wv^T natural via stationary x^T chunks; v bias bv added during the
    PSUM->SBUF copy (DVE tensor_tensor with a broadcast bv tile)
  - scores window [128q,128k]: 4 e-chunk matmuls + one K=9 matmul that adds
    both the off-block -100 mask (rank 9: -100*1x1 + 100*sum u_b x u_b) and
    the key-side bias t[k] (folded into the rank-1 row as t[k]-100)
  - softmax: ACT Exp with fused accum row-sum (off-block entries exp to 0,
    so no explicit mask or max-subtraction is needed; logits are O(1)),
    DVE reciprocal + per-row scale -> bf16
  - attn^T: single DVE 32x32-block transpose == exact transpose of the
    block-diagonal attn (16-blocks lie inside diagonal 32-blocks; off-diag
    32-blocks are exactly 0)
  - out = attnT.T @ v' single K=128 matmul -> PSUM -> copy -> DMA out fp32
"""

import math
import sys

if "/opt/trn_rl_repo" not in sys.path:
    sys.path.insert(0, "/opt/trn_rl_repo")

import numpy as np
import ml_dtypes

BF16 = ml_dtypes.bfloat16
MASK_C = 100.0  # off-block logit penalty; exp(x - 100) flushes to 0 in fp32
BLOCK = 16  # attention block size (ceil(S**(2/3)) blocks => 16 for S=4096)


def _build_nc(S: int, E: int):
    from contextlib import ExitStack

    import concourse.bass as bass
    import concourse.tile as tile
    from concourse import bacc, mybir

    f32 = mybir.dt.float32
    bf16 = mybir.dt.bfloat16
    P = 128
    GROUP = 512  # rows per group
    G = S // GROUP
    NCH = E // P  # e-chunks (4)
    NW = GROUP // P  # windows per group (4)
    NB = P // BLOCK  # 16-blocks per window (8)
    assert S % GROUP == 0 and E == 512

    nc = bacc.Bacc("TRN2", debug=False)

    x_dram = [
        nc.dram_tensor("x1t", [E, S], bf16, kind="ExternalInput").ap(),
        nc.dram_tensor("x2t", [E, S], bf16, kind="ExternalInput").ap(),
    ]
    at_dram = nc.dram_tensor("at", [E, E], bf16, kind="ExternalInput").ap()
    wvt_dram = nc.dram_tensor("wvt", [E, E], bf16, kind="ExternalInput").ap()
    # per-(state, group) post-exp factor M[q,k] = e^{t[k]} * [q,k same block]
    # (softmax(s+t[k]) = exp(s)*e^{t[k]} / rowsum — mask and key-bias applied
    # multiplicatively after exp, host-computed, so no mask/bias matmuls)
    mf_dram = nc.dram_tensor("mfac", [2, G, P, GROUP], f32, kind="ExternalInput").ap()
    bvb_dram = nc.dram_tensor("bvb", [P, E], f32, kind="ExternalInput").ap()
    out_dram = [
        nc.dram_tensor("out1", [S, E], f32, kind="ExternalOutput").ap(),
        nc.dram_tensor("out2", [S, E], f32, kind="ExternalOutput").ap(),
    ]

    Exp = mybir.ActivationFunctionType.Exp

    with ExitStack() as ctx:
        tc = ctx.enter_context(tile.TileContext(nc))

        consts = ctx.enter_context(tc.tile_pool(name="consts", bufs=1))
        xt_pool = ctx.enter_context(tc.tile_pool(name="xt", bufs=2))
        z_pool = ctx.enter_context(tc.tile_pool(name="z", bufs=2))
        v_pool = ctx.enter_context(tc.tile_pool(name="v", bufs=2))
        mf_pool = ctx.enter_context(tc.tile_pool(name="mf", bufs=2))
        sm_pool = ctx.enter_context(tc.tile_pool(name="sm", bufs=3))
        o_pool = ctx.enter_context(tc.tile_pool(name="o", bufs=3))
        psA = ctx.enter_context(tc.tile_pool(name="psA", bufs=3, space="PSUM"))
        psS = ctx.enter_context(tc.tile_pool(name="psS", bufs=3, space="PSUM"))
        psO = ctx.enter_context(tc.tile_pool(name="psO", bufs=2, space="PSUM"))

        # --- constants (batched DMAs; at first — first z matmul needs it) ---
        at_t = consts.tile([P, NCH * E], bf16, name="att", tag="att")
        nc.sync.dma_start(
            at_t.rearrange("p (c e) -> p c e", c=NCH),
            at_dram.rearrange("(c p) e -> p c e", p=P),
        )
        wv_t = consts.tile([P, NCH * E], bf16, name="wvt", tag="wvt")
        nc.scalar.dma_start(
            wv_t.rearrange("p (c e) -> p c e", c=NCH),
            wvt_dram.rearrange("(c p) e -> p c e", p=P),
        )
        bvb_t = consts.tile([P, E], f32, name="bvb", tag="bvb")
        nc.scalar.dma_start(bvb_t[:], bvb_dram[:])

        def at_c(c):  # A^T chunk c: [128 e_in, 512 e_out]
            return at_t[:, c * E : (c + 1) * E]

        def wv_c(c):
            return wv_t[:, c * E : (c + 1) * E]

        # --- group loop (sequential emission measured fastest: a one-group
        # software-pipelined emission order was tried and cost +16us) ---
        st = {}  # per-group state: (xt, zt, vt, mf)

        def emit_load_proj(g):
            r0 = g * GROUP
            xt = {}
            zt = {}
            vt = {}
            mf = {}
            for s in range(2):
                x_tl = xt_pool.tile([P, NCH * GROUP], bf16, name=f"xt{s}", tag=f"xt{s}")
                nc.sync.dma_start(
                    x_tl.rearrange("p (c r) -> p c r", c=NCH),
                    x_dram[s].rearrange("(c p) s -> p c s", p=P)[:, :, r0 : r0 + GROUP],
                )
                xt[s] = x_tl

            def xt_c(s, c):  # x^T chunk c: [128 e_in, 512 rows]
                return xt[s][:, c * GROUP : (c + 1) * GROUP]

            for s in range(2):
                # z_s^T m-chunk [128 e_out, GROUP rows]
                for m in range(NCH):
                    z_ps = psA.tile([P, GROUP], f32, name="zps", tag="psA")
                    for c in range(NCH):
                        nc.tensor.matmul(
                            z_ps[:], at_c(c)[:, m * P : (m + 1) * P], xt_c(s, c),
                            start=(c == 0), stop=(c == NCH - 1),
                        )
                    z_sb = z_pool.tile([P, GROUP], bf16, name=f"zsb{s}{m}", tag=f"zsb{s}{m}")
                    nc.scalar.copy(z_sb[:], z_ps[:])
                    zt[s, m] = z_sb

                # v'_s r-chunk [128 rows, E] = x @ Wv^T (bv is added later,
                # during the normalized out-copy)
                for r in range(NW):
                    v_ps = psA.tile([P, E], f32, name="vps", tag="psA")
                    for c in range(NCH):
                        nc.tensor.matmul(
                            v_ps[:], xt_c(s, c)[:, r * P : (r + 1) * P], wv_c(c),
                            start=(c == 0), stop=(c == NCH - 1),
                        )
                    v_sb = v_pool.tile([P, E], bf16, name=f"vsb{s}{r}", tag=f"vsb{s}{r}")
                    nc.scalar.copy(v_sb[:], v_ps[:])
                    vt[s, r] = v_sb

            # post-exp factor tiles — emitted after the projections so these
            # loads don't compete with the critical x/at transfers
            for s in range(2):
                mf_tl = mf_pool.tile([P, GROUP], f32, name=f"mf{s}", tag=f"mf{s}")
                nc.sync.dma_start(mf_tl[:], mf_dram[s, g])
                mf[s] = mf_tl
            st[g] = (xt, zt, vt, mf)

        def emit_attn(g):
            r0 = g * GROUP
            xt, zt, vt, mf = st.pop(g)
            for w in range(NW):
                ws = slice(w * P, (w + 1) * P)
                for qs, ks in ((0, 1), (1, 0)):
                    s_ps = psS.tile([P, P], f32, name="sps", tag="psS")
                    for m in range(NCH):
                        nc.tensor.matmul(
                            s_ps[:],
                            xt[qs][:, m * GROUP + w * P : m * GROUP + (w + 1) * P],
                            zt[ks, m][:, ws],
                            start=(m == 0), stop=(m == NCH - 1),
                        )
                    exp_sb = sm_pool.tile([P, P], f32, name="expsb", tag="expsb")
                    nc.scalar.activation(exp_sb[:], s_ps[:], Exp)
                    # masked UNNORMALIZED attn = exp * M (zeroes off-block,
                    # applies e^{t[k]}), fused row-sum in the same DVE op;
                    # normalization happens per-q-row on the out-copy below
                    mskd = sm_pool.tile([P, P], bf16, name="mskd", tag="mskd")
                    rsum = sm_pool.tile([P, 1], f32, name="rsum", tag="rsum")
                    nc.vector.scalar_tensor_tensor(
                        mskd[:], exp_sb[:], 1.0, mf[ks][:, ws],
                        op0=mybir.AluOpType.mult, op1=mybir.AluOpType.mult,
                        accum_out=rsum[:],
                    )
                    rcp = sm_pool.tile([P, 1], f32, name="rcp", tag="rcp")
                    nc.vector.reciprocal(rcp[:], rsum[:])
                    attnT = sm_pool.tile([P, P], bf16, name="attnT", tag="attnT")
                    nc.vector.transpose(attnT[:], mskd[:])

                    o_ps = psO.tile([P, E], f32, name="ops", tag="psO")
                    nc.tensor.matmul(o_ps[:], attnT[:], vt[ks, w][:], start=True, stop=True)
                    # out = (attn_unnorm @ v) * recip[q] + bv  — one DVE op
                    o_sb = o_pool.tile([P, E], f32, name=f"osb{qs}", tag=f"osb{qs}")
                    nc.vector.scalar_tensor_tensor(
                        o_sb[:], o_ps[:], rcp[:], bvb_t[:],
                        op0=mybir.AluOpType.mult, op1=mybir.AluOpType.add,
                    )
                    nc.gpsimd.dma_start(out_dram[qs][r0 + w * P : r0 + (w + 1) * P, :], o_sb[:])

        for g in range(G):
            emit_load_proj(g)
            emit_attn(g)

    nc.compile()
    return nc


def _host_inputs(state1, state2, Wq, bq, Wk, bk, Wv, bv, S, E):
    """Build the per-core common (weight) arrays + per-core x arrays."""
    P = 128
    GROUP = 512
    NCH = E // P
    NB = P // BLOCK
    G = S // GROUP
    scale = math.sqrt(E)
    Wq64 = np.asarray(Wq, np.float64)
    Wk64 = np.asarray(Wk, np.float64)
    # A = Wq^T Wk / scale ; device needs A^T = Wk^T Wq / scale  [e_in, e_out]
    at = (Wk64.T @ Wq64 / scale).astype(BF16)
    cvec = (Wk64.T @ np.asarray(bq, np.float64) / scale).astype(np.float32)  # [E]
    wvt = np.ascontiguousarray(np.asarray(Wv, np.float32).T).astype(BF16)
    bvb = np.broadcast_to(np.asarray(bv, np.float32).reshape(1, E), (P, E))
    common = {
        "at": np.ascontiguousarray(at),
        "wvt": wvt,
        "bvb": np.ascontiguousarray(bvb),
    }
    # post-exp factor M[q, k] = [q, k in same 16-block] * e^{t[k]}
    idx = np.arange(P)
    kidx = np.arange(GROUP) % P
    pattern = (idx[:, None] // BLOCK == kidx[None, :] // BLOCK).astype(np.float32)
    x1 = np.asarray(state1, np.float32)
    x2 = np.asarray(state2, np.float32)
    B = x1.shape[0]
    per_core = []
    for b in range(B):
        mfac = np.empty((2, G, P, GROUP), np.float32)
        for s, x in ((0, x1[b]), (1, x2[b])):
            et = np.exp(x @ cvec).reshape(G, 1, GROUP)
            mfac[s] = pattern[None, :, :] * et
        per_core.append(
            {
                "x1t": np.ascontiguousarray(x1[b].T).astype(BF16),
                "x2t": np.ascontiguousarray(x2[b].T).astype(BF16),
                "mfac": mfac,
                **common,
            }
        )
    return per_core


_NC_CACHE = {}


def _get_nc(S, E):
    key = (S, E)
    if key not in _NC_CACHE:
        _NC_CACHE[key] = _build_nc(S, E)
    return _NC_CACHE[key]


def kernel(state1, state2, Wq, bq, Wk, bk, Wv, bv):
    from concourse.bass_utils import run_bass_kernel_spmd

    state1 = np.asarray(state1)
    B, S, E = state1.shape
    assert (B, S, E) == (8, 4096, 512), (B, S, E)

    nc = _get_nc(S, E)
    in_maps = _host_inputs(state1, state2, Wq, bq, Wk, bk, Wv, bv, S, E)
    res = run_bass_kernel_spmd(nc, in_maps, list(range(B)))
    out1 = np.stack([res.results[b]["out1"] for b in range(B)])
    out2 = np.stack([res.results[b]["out2"] for b in range(B)])
    return out1, out2


if __name__ == "__main__":
    rng = np.random.default_rng(0)
    B, S, E = 8, 4096, 512
    ins = {
        "state1": rng.standard_normal((B, S, E), np.float32),
        "state2": rng.standard_normal((B, S, E), np.float32),
        "Wq": rng.standard_normal((E, E), np.float32) * 0.02,
        "bq": rng.standard_normal((E,), np.float32) * 0.02,
        "Wk": rng.standard_normal((E, E), np.float32) * 0.02,
        "bk": rng.standard_normal((E,), np.float32) * 0.02,
        "Wv": rng.standard_normal((E, E), np.float32) * 0.02,
        "bv": rng.standard_normal((E,), np.float32) * 0.02,
    }
    o1, o2 = kernel(**ins)
    print("ok", o1.shape, o2.shape, o1.dtype)



# revision 4
# speedup vs baseline: 1.0642x; 1.0642x over previous
"""Bass/Trainium2 kernel for blockwise cross-attention.

Math (per batch element b, per 16-row block):
  out1 = softmax(q1 k2^T / sqrt(E)) @ v2,  out2 = softmax(q2 k1^T / sqrt(E)) @ v1
with q = x Wq^T + bq etc.  Softmax is shift-invariant along the key axis, so
the q-side bias drops and
  softmax(q1 k2^T / s) == softmax(x1 (A x2^T) + 1 (x2 c)^T)
with A^T = Wk^T Wq / s ("at") and c = Wk^T bq / s precomputed on the host.
z = x at serves as the KEY-side features for both directions, x itself is the
QUERY side.  The v bias is added on the host (out += bv) since softmax rows
sum to 1.

Precision/performance scheme (vs the previous all-bf16 version):
  - z-projection runs in fp8 (e4m3) with DoubleRow perf mode: K=256 per
    matmul at 0.5 cycles/row -> 4x bf16 MAC throughput.  A is scaled by
    S_A=512 on the host so it clears the fp8 normal range; the PSUM->SBUF
    copy divides it back out (exact power of two).
  - everything else runs in fp16 (same 1.0 cycle/row as bf16, ~8x less
    quantization error): z storage, scores matmuls, v-projection, attn
    weights, out matmul, out storage (host converts to fp32).
  - the off-block mask (-100) and key-side bias t[k] = (x c)[k] are added
    PRE-exp by one K=10 fp16 matmul into the scores PSUM:
      row0: 1 x t[k];  row1: (-1) x 100;  rows 2+b: 10*u_b[q] x 10*u_b[k]
    (on-block: +100-100+t, off-block: t-100 -> exp underflows to exactly 0),
    so no post-exp mask multiply is needed and ACT Exp computes the row-sum
    via its fused accumulator.
  - PSUM->SBUF copy budget (the non-PE bottleneck): z and v psums are paired
    into [128,1024] (2-bank) tiles so one ACT copy moves both; the out copy
    runs on DVE with the softmax normalization fused in as a per-partition
    rcp[q] scale; exp+rowsum on ACT; reciprocal + 32x32 block transpose
    (exact for block-diagonal attn) on DVE.

Engine budget per 512-row group (8 groups): PE ~12.4us/group (99us total) is
the bottleneck; ACT ~9.7us, DVE ~8.4us, DMA ~8us/group over sync+gpsimd.

Sharding: pure data-parallel - batch B=8, one batch element per NeuronCore.
"""

import math
import sys

if "/opt/trn_rl_repo" not in sys.path:
    sys.path.insert(0, "/opt/trn_rl_repo")

import numpy as np
import ml_dtypes

F8 = ml_dtypes.float8_e4m3
F16 = np.float16
BLOCK = 16  # attention block size (ceil(S**(2/3)) blocks => 16 for S=4096)
S_A = 512.0  # host scale on A so fp8 holds it; divided out in the z copy


def _build_nc(S: int, E: int):
    from contextlib import ExitStack

    import concourse.bass as bass
    import concourse.tile as tile
    from concourse import bacc, mybir

    f32 = mybir.dt.float32
    f16 = mybir.dt.float16
    f8 = mybir.dt.float8e4
    P = 128
    GROUP = 512  # rows per group
    G = S // GROUP
    NCH = E // P  # e-chunks (4)
    NW = GROUP // P  # windows per group (4)
    MK = 10  # mask matmul contraction size
    assert S % GROUP == 0 and E == 512

    nc = bacc.Bacc("TRN2", debug=False)

    x16_dram = [
        nc.dram_tensor("x1t16", [E, S], f16, kind="ExternalInput").ap(),
        nc.dram_tensor("x2t16", [E, S], f16, kind="ExternalInput").ap(),
    ]
    x8_dram = [
        nc.dram_tensor("x1t8", [E, S], f8, kind="ExternalInput").ap(),
        nc.dram_tensor("x2t8", [E, S], f8, kind="ExternalInput").ap(),
    ]
    at8_dram = nc.dram_tensor("at8", [E, E], f8, kind="ExternalInput").ap()
    wvt_dram = nc.dram_tensor("wvt", [E, E], f16, kind="ExternalInput").ap()
    mskl_dram = nc.dram_tensor("mskl", [MK, P], f16, kind="ExternalInput").ap()
    mskr_dram = nc.dram_tensor("mskr", [2, G, MK, GROUP], f16, kind="ExternalInput").ap()
    out_dram = [
        nc.dram_tensor("out1", [S, E], f16, kind="ExternalOutput").ap(),
        nc.dram_tensor("out2", [S, E], f16, kind="ExternalOutput").ap(),
    ]

    Exp = mybir.ActivationFunctionType.Exp
    DR = mybir.MatmulPerfMode.DoubleRow

    with ExitStack() as ctx:
        tc = ctx.enter_context(tile.TileContext(nc))

        consts = ctx.enter_context(tc.tile_pool(name="consts", bufs=1))
        xt_pool = ctx.enter_context(tc.tile_pool(name="xt", bufs=2))
        x8_pool = ctx.enter_context(tc.tile_pool(name="x8", bufs=2))
        z_pool = ctx.enter_context(tc.tile_pool(name="z", bufs=2))
        v_pool = ctx.enter_context(tc.tile_pool(name="v", bufs=2))
        mk_pool = ctx.enter_context(tc.tile_pool(name="mk", bufs=2))
        sm_pool = ctx.enter_context(tc.tile_pool(name="sm", bufs=3))
        o_pool = ctx.enter_context(tc.tile_pool(name="o", bufs=2))
        psA = ctx.enter_context(tc.tile_pool(name="psA", bufs=2, space="PSUM"))
        psS = ctx.enter_context(tc.tile_pool(name="psS", bufs=2, space="PSUM"))
        psO = ctx.enter_context(tc.tile_pool(name="psO", bufs=2, space="PSUM"))

        # --- constants (at8 first - first z matmul needs it) ---
        at8_t = consts.tile([P, NCH, E], f8, name="at8t", tag="at8t")
        nc.sync.dma_start(at8_t[:], at8_dram.rearrange("(c p) e -> p c e", p=P))
        wv_t = consts.tile([P, NCH, E], f16, name="wvt", tag="wvt")
        nc.scalar.dma_start(wv_t[:], wvt_dram.rearrange("(c p) e -> p c e", p=P))
        mskl_t = consts.tile([MK, P], f16, name="mskl", tag="mskl")
        nc.scalar.dma_start(mskl_t[:], mskl_dram[:])

        # --- group loop ---
        st = {}  # per-group state: (xt, x8, zt, vt, mk)

        def emit_load_proj(g):
            r0 = g * GROUP
            xt = {}
            x8 = {}
            zt = {}
            vt = {}
            mk = {}
            for s in range(2):
                x_tl = xt_pool.tile([P, NCH, GROUP], f16, name=f"xt{s}", tag=f"xt{s}")
                nc.sync.dma_start(
                    x_tl[:],
                    x16_dram[s].rearrange("(c p) s -> p c s", p=P)[:, :, r0 : r0 + GROUP],
                )
                xt[s] = x_tl
                x8_tl = x8_pool.tile([P, NCH, GROUP], f8, name=f"x8{s}", tag=f"x8{s}")
                nc.sync.dma_start(
                    x8_tl[:],
                    x8_dram[s].rearrange("(c p) s -> p c s", p=P)[:, :, r0 : r0 + GROUP],
                )
                x8[s] = x8_tl

            for s in range(2):
                # z_s^T m-chunks: 2 fp8 DoubleRow matmuls each; pairs (m, m+1)
                # share a 2-bank psum so ONE ACT copy moves both to SBUF
                for mh in range(NCH // 2):
                    z_ps = psA.tile([P, 2, GROUP], f32, name="zps", tag="psA")
                    for mi in range(2):
                        for c2 in range(NCH // 2):
                            nc.tensor.matmul(
                                z_ps[:, mi, :],
                                at8_t[:, 2 * c2 : 2 * c2 + 2, (2 * mh + mi) * P : (2 * mh + mi + 1) * P],
                                x8[s][:, 2 * c2 : 2 * c2 + 2, :],
                                start=(c2 == 0), stop=(c2 == NCH // 2 - 1),
                                perf_mode=DR,
                            )
                    z_sb = z_pool.tile([P, 2, GROUP], f16, name=f"zsb{s}{mh}", tag=f"zsb{s}{mh}")
                    nc.scalar.mul(z_sb[:], z_ps[:], 1.0 / S_A)
                    zt[s, mh] = z_sb

                # v'_s r-chunks [128 rows, E] = x @ Wv^T, r-pairs share a
                # 2-bank psum -> one ACT copy (bv is added on the host)
                for rh in range(NW // 2):
                    v_ps = psA.tile([P, 2, E], f32, name="vps", tag="psA")
                    for ri in range(2):
                        r = 2 * rh + ri
                        for c in range(NCH):
                            nc.tensor.matmul(
                                v_ps[:, ri, :], xt[s][:, c, r * P : (r + 1) * P], wv_t[:, c, :],
                                start=(c == 0), stop=(c == NCH - 1),
                            )
                    v_sb = v_pool.tile([P, 2, E], f16, name=f"vsb{s}{rh}", tag=f"vsb{s}{rh}")
                    nc.scalar.copy(v_sb[:], v_ps[:])
                    vt[s, rh] = v_sb

            # mask/bias rhs tiles - emitted after the critical x/at transfers
            for s in range(2):
                mk_tl = mk_pool.tile([MK, GROUP], f16, name=f"mk{s}", tag=f"mk{s}")
                nc.sync.dma_start(mk_tl[:], mskr_dram[s, g])
                mk[s] = mk_tl
            st[g] = (xt, x8, zt, vt, mk)

        def emit_attn(g):
            xt, x8, zt, vt, mk = st.pop(g)
            o_sb = {}
            for s in range(2):
                o_sb[s] = o_pool.tile([P, NW, E], f16, name=f"osb{s}", tag=f"osb{s}")
            for w in range(NW):
                ws = slice(w * P, (w + 1) * P)
                for qs, ks in ((0, 1), (1, 0)):
                    s_ps = psS.tile([P, P], f32, name="sps", tag="psS")
                    for m in range(NCH):
                        nc.tensor.matmul(
                            s_ps[:],
                            xt[qs][:, m, ws],
                            zt[ks, m // 2][:, m % 2, ws],
                            start=(m == 0), stop=False,
                        )
                    # adds t[k] - 100*offblock(q,k) pre-exp; off-block entries
                    # then exp-underflow to exactly 0 (no post-exp masking)
                    nc.tensor.matmul(
                        s_ps[:], mskl_t[:], mk[ks][:, ws], start=False, stop=True,
                    )
                    exp_sb = sm_pool.tile([P, P], f16, name="expsb", tag="expsb")
                    rsum = sm_pool.tile([P, 1], f32, name="rsum", tag="rsum")
                    nc.scalar.activation(exp_sb[:], s_ps[:], Exp, accum_out=rsum[:])
                    rcp = sm_pool.tile([P, 1], f32, name="rcp", tag="rcp")
                    nc.vector.reciprocal(rcp[:], rsum[:])
                    # 32x32 block transpose == exact transpose of the
                    # block-diagonal attn (off-diagonal 32-blocks are 0)
                    attnT = sm_pool.tile([P, P], f16, name="attnT", tag="attnT")
                    nc.vector.transpose(attnT[:], exp_sb[:])

                    o_ps = psO.tile([P, E], f32, name="ops", tag="psO")
                    nc.tensor.matmul(o_ps[:], attnT[:], vt[ks, w // 2][:, w % 2, :], start=True, stop=True)
                    # out = (attn_unnorm @ v) * recip[q]; normalization fused
                    # into the PSUM->SBUF copy as a per-partition scale (DVE)
                    nc.vector.tensor_scalar(
                        o_sb[qs][:, w, :], o_ps[:], rcp[:], None, mybir.AluOpType.mult,
                    )
            for s in range(2):
                nc.gpsimd.dma_start(
                    out_dram[s].rearrange("(g w p) e -> g p w e", w=NW, p=P)[g],
                    o_sb[s][:],
                )

        for g in range(G):
            emit_load_proj(g)
            emit_attn(g)

    nc.compile()
    return nc


def _host_inputs(state1, state2, Wq, bq, Wk, bk, Wv, bv, S, E):
    """Build the per-core common (weight) arrays + per-core x arrays."""
    P = 128
    GROUP = 512
    G = S // GROUP
    MK = 10
    scale = math.sqrt(E)
    Wq64 = np.asarray(Wq, np.float64)
    Wk64 = np.asarray(Wk, np.float64)
    # z = x @ at with at = Wk^T Wq / scale;  scores12 = x1 @ z2^T + t2[k]
    at = Wk64.T @ Wq64 / scale
    at8 = np.ascontiguousarray((at * S_A).astype(F8))
    cvec = (Wk64.T @ np.asarray(bq, np.float64) / scale).astype(np.float32)  # [E]
    wvt = np.ascontiguousarray(np.asarray(Wv, np.float32).T).astype(F16)
    # mask/bias matmul parts: lhsT [10, 128] (q side), rhs [10, GROUP] (k side)
    qidx = np.arange(P) // BLOCK % 8
    mskl = np.zeros((MK, P), np.float32)
    mskl[0, :] = 1.0
    mskl[1, :] = -1.0
    for b_ in range(8):
        mskl[2 + b_, qidx == b_] = 10.0
    mskl = mskl.astype(F16)
    kpat = np.zeros((MK, GROUP), np.float32)
    kpat[1, :] = 100.0
    kb = np.arange(GROUP) % P // BLOCK
    for b_ in range(8):
        kpat[2 + b_, kb == b_] = 10.0
    common = {"at8": at8, "wvt": wvt, "mskl": mskl}
    x1 = np.asarray(state1, np.float32)
    x2 = np.asarray(state2, np.float32)
    B = x1.shape[0]
    per_core = []
    for b in range(B):
        mskr = np.broadcast_to(kpat, (2, G, MK, GROUP)).copy()
        for s, x in ((0, x1[b]), (1, x2[b])):
            mskr[s, :, 0, :] = (x @ cvec).reshape(G, GROUP)
        per_core.append(
            {
                "x1t16": np.ascontiguousarray(x1[b].T).astype(F16),
                "x2t16": np.ascontiguousarray(x2[b].T).astype(F16),
                "x1t8": np.ascontiguousarray(x1[b].T).astype(F8),
                "x2t8": np.ascontiguousarray(x2[b].T).astype(F8),
                "mskr": mskr.astype(F16),
                **common,
            }
        )
    return per_core


_NC_CACHE = {}


def _get_nc(S, E):
    key = (S, E)
    if key not in _NC_CACHE:
        _NC_CACHE[key] = _build_nc(S, E)
    return _NC_CACHE[key]


def kernel(state1, state2, Wq, bq, Wk, bk, Wv, bv):
    from concourse.bass_utils import run_bass_kernel_spmd

    state1 = np.asarray(state1)
    B, S, E = state1.shape
    assert (B, S, E) == (8, 4096, 512), (B, S, E)

    nc = _get_nc(S, E)
    in_maps = _host_inputs(state1, state2, Wq, bq, Wk, bk, Wv, bv, S, E)
    res = run_bass_kernel_spmd(nc, in_maps, list(range(B)))
    bvf = np.asarray(bv, np.float32)
    out1 = np.stack([res.results[b]["out1"].astype(np.float32) + bvf for b in range(B)])
    out2 = np.stack([res.results[b]["out2"].astype(np.float32) + bvf for b in range(B)])
    return out1, out2


if __name__ == "__main__":
    rng = np.random.default_rng(0)
    B, S, E = 8, 4096, 512
    ins = {
        "state1": rng.standard_normal((B, S, E), np.float32),
        "state2": rng.standard_normal((B, S, E), np.float32),
        "Wq": rng.standard_normal((E, E), np.float32) * 0.02,
        "bq": rng.standard_normal((E,), np.float32) * 0.02,
        "Wk": rng.standard_normal((E, E), np.float32) * 0.02,
        "bk": rng.standard_normal((E,), np.float32) * 0.02,
        "Wv": rng.standard_normal((E, E), np.float32) * 0.02,
        "bv": rng.standard_normal((E,), np.float32) * 0.02,
    }
    o1, o2 = kernel(**ins)
    print("ok", o1.shape, o2.shape, o1.dtype)
